# revision 24
# baseline (speedup 1.0000x reference)
"""Trainium2 Bass kernel for nn_ExpertFFNEnsemble (MoE routing, 8 experts, top-2).

Strategy: data-parallel over tokens (8192 tokens -> 1024/core, 8 cores).
v4 restructure (v3 lessons: ACT table thrash, scalar-ring congestion,
PE-FIFO coupling of rank chain with shared-fc1):
  - phase A: all router tiles first (PE-light, DVE-driven), sigmoids
    BATCHED into 2 ACT ops (no sigmoid<->gelu table reloads), then
    rank/compaction+scatters, idxw readback + dispatch gathers on the
    gpsimd ring, THEN shared fc1 (58 us of PE) overlapping the gathers
  - expert phase: serial expert pairs; fc2 processes both d-halves per
    weight chunk so one hT LDWEIGHTS feeds two matmuls; w2 streams on
    the gpsimd ring (ACT only does fc1 gelu); 32-row fc2 leftovers pack
    2-way per j into one PSUM bank via tile_position col tiling
    (zero-matmul opens the bank, leftovers accumulate with start=False)
  - shared-expert fc2 LAST (sw2 SBUF-resident, j-outer) covering the
    per-tile combine + LayerNorm tail; LN uses DVE bn_stats/bn_aggr
    (mean+var in one pass, no ACT tables), ys scale on DVE
No cross-core communication; host shards tokens / packs weights and
concatenates per-core output slices.
"""

import sys

sys.path.insert(0, "/opt/trn_rl_repo")

import numpy as np
import ml_dtypes

import concourse.bass as bass
import concourse.mybir as mybir
import concourse.tile as tile
from concourse import bacc
from concourse.bass import IndirectOffsetOnAxis
from concourse.bass_utils import run_bass_kernel_spmd

P = 128
B, S, D, F = 4, 2048, 1024, 4096
F2 = F // 2
E = 8
NCORES = 8
T = (B * S) // NCORES           # 1024 tokens per core
NT = T // P                     # 8 token tiles
ND = D // P                     # 8 d-chunks
NF = F // P                     # 32 f-chunks
NF2 = F2 // P                   # 16 f2-chunks
CAP = 288                       # per-expert token capacity (2.25 x 128)
NIDX = E * CAP                  # 2304 bucket rows (%128 == 0)
NCH = CAP // 16                 # idx columns per expert (18)
TRASH = NIDX                    # overflow-redirect row
GCAP = 384                      # gather width per expert (3 x 128, padded)
NIDXG = (E - 1) * CAP + GCAP    # bid rows covered by padded gathers (2400)
NW = NIDXG // 16                # idxw row width (150)
BID_ROWS = ((NIDXG + P + 127) // P) * P   # bucket-id rows incl. trash, %128
YBK_ROWS = NIDX + P             # fc2 output rows incl. trash region
LN_EPS = 1e-5
FT = mybir.ActivationFunctionType
dt = mybir.dt
AX = mybir.AxisListType
OP = mybir.AluOpType

_PROGRAM = None


def _build_consts(nc, pp, pA, aps):
    c = {}
    c["iota8"] = pp.tile([P, 8], dt.float32, name="iota8")
    nc.scalar.dma_start(c["iota8"][:], aps["iota8"][:])
    c["tri"] = pp.tile([P, P], dt.bfloat16, name="tri")
    nc.scalar.dma_start(c["tri"][:], aps["tri"][:])
    c["ident"] = pp.tile([P, P], dt.float32, name="ident")
    nc.scalar.dma_start(c["ident"][:], aps["ident"][:])
    c["onesb"] = pp.tile([1, P], dt.bfloat16, name="onesb")
    nc.scalar.dma_start(c["onesb"][:], aps["onesb"][:])
    c["onesf"] = pp.tile([1, P], dt.float32, name="onesf")
    nc.scalar.dma_start(c["onesf"][:], aps["onesf"][:])
    c["tri8f"] = pp.tile([8, 8], dt.float32, name="tri8f")
    nc.scalar.dma_start(c["tri8f"][:], aps["tri8f"][:])
    c["onescolb"] = pp.tile([P, 1], dt.bfloat16, name="onescolb")
    nc.scalar.dma_start(c["onescolb"][:], aps["onescolb"][:])
    c["eps_t"] = pp.tile([P, 1], dt.float32, name="eps_t")
    nc.vector.memset(c["eps_t"][:], LN_EPS)

    c["gate_sb"] = pp.tile([P, E, NF], dt.float32, name="gate_sb")
    nc.scalar.dma_start(c["gate_sb"][:], aps["gate"][:])
    c["b1g_sb"] = pp.tile([P, E, NF], dt.float32, name="b1g_sb")
    b1_tmp = pA.tile([P, E, NF], dt.float32, tag="b1_tmp")
    nc.scalar.dma_start(b1_tmp[:], aps["b1"][:])
    nc.vector.tensor_mul(c["b1g_sb"][:], b1_tmp[:], c["gate_sb"][:])
    c["sg_sb"] = pp.tile([P, NF2], dt.float32, name="sg_sb")
    nc.scalar.dma_start(c["sg_sb"][:], aps["sgate"][:])
    c["sb1g_sb"] = pp.tile([P, NF2], dt.float32, name="sb1g_sb")
    sb1_tmp = pA.tile([P, NF2], dt.float32, tag="sb1_tmp")
    nc.scalar.dma_start(sb1_tmp[:], aps["sb1"][:])
    nc.vector.tensor_mul(c["sb1g_sb"][:], sb1_tmp[:], c["sg_sb"][:])

    # zero-init internal DRAM (bucket ids + ybk trash rows)
    zid = pA.tile([P, BID_ROWS // P], dt.int16, tag="zid")
    nc.vector.memset(zid[:], 0)
    nc.sync.dma_start(
        aps["bid"].rearrange("(p c) one -> p (c one)", p=P), zid[:])
    ztrash = pA.tile([P, D], dt.bfloat16, tag="ztrash")
    nc.vector.memset(ztrash[:], 0.0)
    nc.sync.dma_start(aps["ybk"][TRASH:TRASH + P, :], ztrash[:])

    c["sig_bc"] = pp.tile([P, 1], dt.float32, name="sig_bc")
    c["iotat_sb"] = pp.tile([P, NT], dt.int16, name="iotat_sb")
    nc.scalar.dma_start(c["iotat_sb"][:], aps["iotat"][:])
    return c


def _router_tile(nc, pA, pAps, pp, c, st, xtf, i):
    """Router logits + top-2 for token tile i (sigmoids deferred)."""
    pl = pAps.tile([P, 8], dt.float32, tag="rtr",
                   name=f"pl{i}", space="PSUM", bufs=2)
    for k in range(ND):
        nc.tensor.matmul(pl[:], lhsT=xtf[:, k, :],
                         rhs=c["rw_sb"][:, k, :],
                         start=(k == 0), stop=False)
    nc.tensor.matmul(pl[:], lhsT=c["onesf"][:, :], rhs=c["rb_sb"][:, :],
                     start=False, stop=True)
    vals = pA.tile([P, 8], dt.float32, tag="vals")
    idx = pA.tile([P, 8], dt.uint32, tag="idx")
    lt = pA.tile([P, 8], dt.float32, tag="lt")
    nc.vector.tensor_copy(lt[:], pl[:])
    nc.vector.max_with_indices(vals[:], idx[:], lt[:])

    nc.vector.tensor_sub(st["d01"][:, i:i + 1], vals[:, 0:1], vals[:, 1:2])

    ef = pA.tile([P, 2], dt.float32, tag=f"ef{i}")
    nc.vector.tensor_copy(ef[:], idx[:, 0:2])
    oh0 = pA.tile([P, 8], dt.float32, tag=f"oh0_{i}")
    oh1 = pA.tile([P, 8], dt.float32, tag=f"oh1_{i}")
    nc.vector.tensor_tensor(
        out=oh0[:], in0=ef[:, 0:1].to_broadcast([P, 8]),
        in1=c["iota8"][:], op=OP.is_equal)
    nc.vector.tensor_tensor(
        out=oh1[:], in0=ef[:, 1:2].to_broadcast([P, 8]),
        in1=c["iota8"][:], op=OP.is_equal)
    A = pA.tile([P, 8], dt.bfloat16, tag=f"A{i}")
    nc.vector.tensor_add(A[:], oh0[:], oh1[:])
    st["A"].append((A, ef, oh0, oh1))


def _compaction_tile(nc, pA, pAps, pp, c, aps, st, i):
    """Rank (serial carry) + bucket position + scatter for tile i."""
    tsl = slice(i * P, (i + 1) * P)
    A, ef, oh0, oh1 = st["A"][i]
    rank_sb, carry = st["rank_sb"], st["carry"]

    pr = pAps.tile([E, P], dt.float32, tag="rtr2",
                   name=f"pr{i}", space="PSUM", bufs=2)
    nc.tensor.matmul(pr[:], lhsT=A[:], rhs=c["tri"][:],
                     start=True, stop=True)
    nc.vector.tensor_scalar_add(rank_sb[:, tsl], pr[:], carry[:, 0:1])
    nc.vector.tensor_copy(carry[:], rank_sb[:, i * P + P - 1:i * P + P])

    prt = pAps.tile([P, E], dt.float32, tag="rtr3",
                    name=f"prt{i}", space="PSUM", bufs=2)
    nc.tensor.transpose(prt[:], rank_sb[:, tsl], c["ident"][:E, :E])
    rank_t = pA.tile([P, E], dt.float32, tag="rank_t")
    nc.scalar.copy(rank_t[:], prt[:])

    tmp = pA.tile([P, 8], dt.float32, tag="tmp")
    r0 = pA.tile([P, 1], dt.float32, tag="r0")
    r1 = pA.tile([P, 1], dt.float32, tag="r1")
    nc.vector.tensor_mul(tmp[:], oh0[:], rank_t[:])
    nc.vector.reduce_sum(r0[:], tmp[:], axis=AX.X)
    nc.vector.tensor_mul(tmp[:], oh1[:], rank_t[:])
    nc.vector.reduce_sum(r1[:], tmp[:], axis=AX.X)

    posf = pA.tile([P, 2], dt.float32, tag="posf")
    nc.vector.tensor_scalar(out=posf[:, 0:1], in0=ef[:, 0:1],
                            scalar1=float(CAP), scalar2=None, op0=OP.mult)
    nc.vector.tensor_scalar(out=posf[:, 1:2], in0=ef[:, 1:2],
                            scalar1=float(CAP), scalar2=None, op0=OP.mult)
    nc.vector.scalar_tensor_tensor(
        out=posf[:, 0:1], in0=r0[:], scalar=-1.0, in1=posf[:, 0:1],
        op0=OP.add, op1=OP.add)
    nc.vector.scalar_tensor_tensor(
        out=posf[:, 1:2], in0=r1[:], scalar=-1.0, in1=posf[:, 1:2],
        op0=OP.add, op1=OP.add)
    ovf = pA.tile([P, 2], dt.uint8, tag="ovf")
    nc.vector.tensor_scalar(out=ovf[:, 0:1], in0=r0[:], scalar1=float(CAP),
                            scalar2=None, op0=OP.is_gt)
    nc.vector.tensor_scalar(out=ovf[:, 1:2], in0=r1[:], scalar1=float(CAP),
                            scalar2=None, op0=OP.is_gt)
    trash = pA.tile([P, 2], dt.float32, tag="trash")
    nc.vector.memset(trash[:], float(TRASH))
    nc.vector.copy_predicated(posf[:], ovf[:], trash[:])
    pos_i = pp.tile([P, 2], dt.int32, tag=f"pos{i}")
    nc.vector.tensor_copy(pos_i[:], posf[:])
    st["pos"].append(pos_i)

    # transposed scatter offsets: sg = (pos % 16) * NW + (pos >> 4), so
    # the idxw readback is contiguous per partition (16-row wrap layout)
    div16 = pA.tile([P, 2], dt.int32, tag="div16")
    nc.vector.tensor_scalar(out=div16[:], in0=pos_i[:], scalar1=4,
                            scalar2=None, op0=OP.arith_shift_right)
    mod16 = pA.tile([P, 2], dt.int32, tag="mod16")
    nc.vector.tensor_scalar(out=mod16[:], in0=pos_i[:], scalar1=15,
                            scalar2=None, op0=OP.bitwise_and)
    sg = pA.tile([P, 2], dt.int32, tag="sg")
    nc.vector.tensor_scalar(out=sg[:], in0=mod16[:], scalar1=NW,
                            scalar2=None, op0=OP.mult)
    nc.vector.tensor_add(sg[:], sg[:], div16[:])
    for s in range(2):
        nc.gpsimd.indirect_dma_start(
            out=aps["bid"][:, :],
            out_offset=IndirectOffsetOnAxis(ap=sg[:, s:s + 1], axis=0),
            in_=c["iotat_sb"][:, i:i + 1], in_offset=None)


def _shared_fc1_group(nc, sw1p, ps3, c, aps, xtb, hsT, cur, m):
    """One (m5, mm) group of the shared fc1: one weight column feeds
    both 512-token halves (shared LDWEIGHTS), 2 psum banks."""
    m5, mm = divmod(m, 4)
    sw1m = cur[m5]
    pms = [ps3.tile([P, 512], dt.float32, tag=f"pm3{n}",
                    name=f"pm3_{m}_{n}", space="PSUM")
           for n in range(2)]
    for k in range(ND):
        for n in range(2):
            nc.tensor.matmul(
                pms[n][:], lhsT=sw1m[:, k, mm * P:(mm + 1) * P],
                rhs=xtb[:, 4 * n:4 * n + 4, k, :],
                start=(k == 0), stop=(k == ND - 1))
    for n in range(2):
        nc.scalar.activation(
            hsT[:, m, n * 512:(n + 1) * 512], pms[n][:], FT.Gelu,
            bias=c["sb1g_sb"][:, m:m + 1],
            scale=c["sg_sb"][:, m:m + 1])


def _fc1_expert(nc, w1p, ps1, c, aps, gxe, hT, e):
    """Expert e fc1: 8 weight chunks x 4 f-tiles, gelu+gate into hT."""
    for m5 in range(F // 512):
        w1m = w1p.tile([P, ND, 512], dt.bfloat16, tag="w1m",
                       name=f"w1m_{e}_{m5}")
        nc.sync.dma_start(w1m[:], aps["w1"][e, m5])
        for mm in range(4):
            m = m5 * 4 + mm
            pm = ps1.tile([P, CAP], dt.float32, tag="pm",
                          name=f"pm_{e}_{m}", space="PSUM")
            for k in range(ND):
                nc.tensor.matmul(
                    pm[:], lhsT=w1m[:, k, mm * P:(mm + 1) * P],
                    rhs=gxe[e][:, k, 0:CAP],
                    start=(k == 0), stop=(k == ND - 1))
            nc.scalar.activation(
                hT[:, m, :], pm[:], FT.Gelu,
                bias=c["b1g_sb"][:, e, m:m + 1],
                scale=c["gate_sb"][:, e, m:m + 1])


def _fc2_pair(nc, w2p, yevp, ps2, psL, c, aps, eA, eB, hTA, hTB):
    """fc2 for an expert pair: both d-halves per weight chunk (one hT
    LDWEIGHTS feeds 2 matmuls); 4 shared full psum banks reused
    eA->eB; 32-row leftovers pack 2-way per j into one shared bank."""
    pysL = psL.tile([P, 512], dt.float32, tag="pysL",
                    name=f"pysL_{eA}", space="PSUM")
    nc.tensor.matmul(pysL[:, :], lhsT=c["z512"][:, 0:P],
                     rhs=c["z512"][:, :], start=True, stop=False)
    hTs = (hTA, hTB)
    for x, e in enumerate((eA, eB)):
        hT = hTs[x]
        pys = {}
        for t in range(2):
            for h in range(2):
                pys[(t, h)] = ps2.tile(
                    [P, 512], dt.float32, tag=f"py{t}{h}",
                    name=f"py_{e}_{t}{h}", space="PSUM")
        for kg in range(NF // 4):
            w2x = w2p.tile([P, 4, 2, 512], dt.bfloat16, tag="w2e",
                           name=f"w2e_{e}_{kg}")
            nc.scalar.dma_start(w2x[:], aps["w2"][e, kg])
            for j in range(4):
                k = kg * 4 + j
                stt = (k == 0)
                for t in range(2):
                    lhsT = hT[:, k, t * P:(t + 1) * P]
                    for h in range(2):
                        nc.tensor.matmul(
                            pys[(t, h)][:], lhsT=lhsT,
                            rhs=w2x[:, j, h, :], start=stt, stop=False)
                lhsTl = hT[:, k, 2 * P:2 * P + 32]
                for h in range(2):
                    lo = x * 64 + h * 32
                    nc.tensor.matmul(
                        pysL[lo:lo + 32, :], lhsT=lhsTl,
                        rhs=w2x[:, j, h, :], start=False, stop=False,
                        tile_position=(0, lo))
        # epilogue: bias matmul + evacuate full tiles of expert e
        for t in range(2):
            for h in range(2):
                hsl = slice(h * 512, (h + 1) * 512)
                nc.tensor.matmul(
                    pys[(t, h)][:], lhsT=c["onesb"][:, :],
                    rhs=c["b2_sb"][:, e, hsl], start=False, stop=True)
                yev = yevp.tile([P, 512], dt.bfloat16, tag="yev",
                                name=f"yev_{e}_{t}{h}")
                nc.vector.tensor_copy(yev[:], pys[(t, h)][:])
                nc.gpsimd.dma_start(
                    aps["ybk"][e * CAP + t * P:e * CAP + (t + 1) * P, hsl],
                    yev[:])
    # leftover bias + evacuation (all four regions live in pysL)
    for x, e in enumerate((eA, eB)):
        for h in range(2):
            hsl = slice(h * 512, (h + 1) * 512)
            lo = x * 64 + h * 32
            last = (x == 1 and h == 1)
            nc.tensor.matmul(
                pysL[lo:lo + 32, :], lhsT=c["onesb"][:, 0:32],
                rhs=c["b2_sb"][:, e, hsl], start=False, stop=last,
                tile_position=(0, lo))
    for x, e in enumerate((eA, eB)):
        yevL = yevp.tile([P, 512], dt.bfloat16, tag="yevL",
                         name=f"yevL_{e}")
        nc.vector.tensor_copy(yevL[:64, :], pysL[x * 64:x * 64 + 64, :])
        for h in range(2):
            hsl = slice(h * 512, (h + 1) * 512)
            nc.gpsimd.dma_start(
                aps["ybk"][e * CAP + 2 * P:e * CAP + 2 * P + 32, hsl],
                yevL[h * 32:h * 32 + 32, :])


def _shared_fc2_combine_ln(nc, tc, pp, c, aps, st, hsT, sw2r):
    """j-outer shared fc2 on PE; gathers + combine + LayerNorm pipelined
    per token tile underneath it. LN stats via DVE bn_stats/bn_aggr."""
    psC = tc.alloc_tile_pool(name="psC", bufs=2, space="PSUM")
    pCg = tc.alloc_tile_pool(name="pCg", bufs=6)
    pC = tc.alloc_tile_pool(name="pC", bufs=2)
    pCw = tc.alloc_tile_pool(name="pCw", bufs=1)

    lng_bc = pCw.tile([P, D], dt.float32, name="lng_bc")
    nc.scalar.dma_start(lng_bc[:], aps["lng"].to_broadcast([P, D]))
    lnb_bc = pCw.tile([P, D], dt.float32, name="lnb_bc")
    nc.scalar.dma_start(lnb_bc[:], aps["lnb"].to_broadcast([P, D]))
    sb2_sb = pCw.tile([1, D], dt.bfloat16, name="sb2_sb")
    nc.scalar.dma_start(sb2_sb[:], aps["sb2"][:, :])

    # issue all moe-output gathers up-front (gpsimd queue drains in order)
    g0s, g1s = [], []
    for i in range(NT):
        g0 = pCg.tile([P, D], dt.bfloat16, tag="g0", name=f"g0_{i}")
        g1 = pCg.tile([P, D], dt.bfloat16, tag="g1", name=f"g1_{i}")
        nc.gpsimd.indirect_dma_start(
            out=g0[:], out_offset=None, in_=aps["ybk"][:, :],
            in_offset=IndirectOffsetOnAxis(ap=st["pos"][i][:, 0:1], axis=0))
        nc.gpsimd.indirect_dma_start(
            out=g1[:], out_offset=None, in_=aps["ybk"][:, :],
            in_offset=IndirectOffsetOnAxis(ap=st["pos"][i][:, 1:2], axis=0))
        g0s.append(g0)
        g1s.append(g1)

    for i in range(NT):
        psY = [psC.tile([P, 512], dt.float32, tag=f"psY{n}",
                        name=f"psY_{i}_{n}", space="PSUM")
               for n in range(2)]
        for k in range(NF2):
            lhsT = hsT[:, k, i * P:(i + 1) * P]
            for n in range(2):
                nc.tensor.matmul(
                    psY[n][:], lhsT=lhsT,
                    rhs=sw2r[:, k, n * 512:(n + 1) * 512],
                    start=(k == 0), stop=False)
        ysi = pC.tile([P, D], dt.bfloat16, tag="ysi", name=f"ysi_{i}")
        for n in range(2):
            nsl = slice(n * 512, (n + 1) * 512)
            nc.tensor.matmul(
                psY[n][:], lhsT=c["onesb"][:, :],
                rhs=sb2_sb[:, nsl], start=False, stop=True)
            nc.scalar.activation(ysi[:, nsl], psY[n][:], FT.Copy,
                                 scale=c["sig_bc"][:, 0:1])

        # combine + LN for tile i, balanced across DVE / ACT / GpSimd
        g0c = pC.tile([P, D], dt.bfloat16, tag="g0c", name=f"g0c_{i}")
        nc.scalar.activation(g0c[:], g0s[i][:], FT.Copy,
                             scale=st["cw0"][:, i:i + 1])
        g1c = pC.tile([P, D], dt.bfloat16, tag="g1c", name=f"g1c_{i}")
        nc.scalar.activation(g1c[:], g1s[i][:], FT.Copy,
                             scale=st["cw1"][:, i:i + 1])
        comb = pC.tile([P, D], dt.bfloat16, tag="comb", name=f"comb_{i}")
        nc.vector.tensor_add(comb[:], g0c[:], ysi[:])
        nc.vector.tensor_add(comb[:], comb[:], g1c[:])
        stat6 = pC.tile([P, 2, 6], dt.float32, tag="stat6", name=f"st6_{i}")
        for n in range(2):
            nc.vector.bn_stats(stat6[:, n, :], comb[:, n * 512:(n + 1) * 512])
        mv = pC.tile([P, 2], dt.float32, tag="mv", name=f"mv_{i}")
        nc.vector.bn_aggr(mv[:], stat6[:])
        sd = pC.tile([P, 1], dt.float32, tag="sd", name=f"sd_{i}")
        nc.scalar.activation(sd[:], mv[:, 1:2], FT.Sqrt,
                             bias=c["eps_t"][:, 0:1])
        rinv = pC.tile([P, 1], dt.float32, tag="rinv", name=f"ri_{i}")
        nc.vector.reciprocal(rinv[:], sd[:])
        yc = pC.tile([P, D], dt.bfloat16, tag="yc", name=f"yc_{i}")
        nc.vector.scalar_tensor_tensor(
            out=yc[:], in0=comb[:], scalar=mv[:, 0:1],
            in1=lng_bc[:], op0=OP.subtract, op1=OP.mult)
        o1 = pC.tile([P, D], dt.float32, tag="o1", name=f"o1_{i}")
        nc.vector.scalar_tensor_tensor(
            out=o1[:], in0=yc[:], scalar=rinv[:, 0:1],
            in1=lnb_bc[:], op0=OP.mult, op1=OP.add)
        nc.sync.dma_start(aps["out"][i * P:(i + 1) * P, :], o1[:])

    pCw.release()
    pC.release()
    pCg.release()
    psC.release()


def build_program():
    nc = bacc.Bacc("TRN2", target_bir_lowering=False, debug=False,
                   num_devices=NCORES)

    def din(name, shape, dtype):
        return nc.dram_tensor(name, list(shape), dtype,
                              kind="ExternalInput").ap()

    aps = {
        "xtf": din("xtf", [P, NT, ND, P], dt.float32),
        "xtb": din("xtb", [P, NT, ND, P], dt.bfloat16),
        "xbf": din("xbf", [T, D], dt.bfloat16),
        "rw": din("rw", [P, ND, E], dt.float32),
        "rb": din("rb", [1, E], dt.float32),
        "w1": din("w1", [E, F // 512, P, ND, 512], dt.bfloat16),
        "w2": din("w2", [E, NF // 4, P, 4, 2, 512], dt.bfloat16),
        "b1": din("b1", [P, E, NF], dt.float32),
        "gate": din("gate", [P, E, NF], dt.float32),
        "b2": din("b2", [E, D], dt.bfloat16),
        "sw1": din("sw1", [F2 // 512, P, ND, 512], dt.bfloat16),
        "sb1": din("sb1", [P, NF2], dt.float32),
        "sgate": din("sgate", [P, NF2], dt.float32),
        "sw2r": din("sw2r", [P, NF2, D], dt.bfloat16),
        "sb2": din("sb2", [1, D], dt.bfloat16),
        "shw": din("shw", [1, 1], dt.float32),
        "lng": din("lng", [1, D], dt.float32),
        "lnb": din("lnb", [1, D], dt.float32),
        "iota8": din("iota8", [P, 8], dt.float32),
        "iotat": din("iotat", [P, NT], dt.int16),
        "tri": din("tri", [P, P], dt.bfloat16),
        "ident": din("ident", [P, P], dt.float32),
        "onesb": din("onesb", [1, P], dt.bfloat16),
        "tri8f": din("tri8f", [8, 8], dt.float32),
        "onescolb": din("onescolb", [P, 1], dt.bfloat16),
        "onesf": din("onesf", [1, P], dt.float32),
    }
    aps["out"] = nc.dram_tensor("out", [T, D], dt.float32,
                                kind="ExternalOutput").ap()
    aps["bid"] = nc.dram_tensor("bid_i", [BID_ROWS, 1], dt.int16).ap()
    aps["ybk"] = nc.dram_tensor("ybk_i", [YBK_ROWS, D], dt.bfloat16).ap()

    with tile.TileContext(nc) as tc:
        # long-lived pools first (stack discipline)
        pp = tc.alloc_tile_pool(name="persist", bufs=1)
        hsTp = tc.alloc_tile_pool(name="hsTp", bufs=1)
        sw2p = tc.alloc_tile_pool(name="sw2p", bufs=1)
        pgx = tc.alloc_tile_pool(name="gx_pool", bufs=1)

        hsT = hsTp.tile([P, NF2, T], dt.bfloat16)
        sw2r = sw2p.tile([P, NF2, D], dt.bfloat16, name="sw2r")
        gxe = {}
        for e in range(5):
            gxe[e] = pgx.tile([P, ND, GCAP], dt.bfloat16, tag=f"gx{e}",
                              name=f"gx{e}")
        idxw = pgx.tile([P, NIDXG // 16], dt.int16)

        # ---- phase A pools
        pxT = tc.alloc_tile_pool(name="xtf_pool", bufs=4)
        pxtb = tc.alloc_tile_pool(name="xtb_pool", bufs=1)
        sw1p = tc.alloc_tile_pool(name="sw1p", bufs=3)
        pA = tc.alloc_tile_pool(name="pA", bufs=2)
        pAps = tc.alloc_tile_pool(name="pAps", bufs=1, space="PSUM")
        ps3 = tc.alloc_tile_pool(name="ps3", bufs=1, space="PSUM")

        # tiny critical DMAs first: shared-weight scalar + router weights
        shw_sb = pA.tile([1, 1], dt.float32, tag="shw", name="shw_sb")
        nc.scalar.dma_start(shw_sb[:], aps["shw"][:, :])
        sig1 = pA.tile([1, 1], dt.float32, tag="sig1", name="sig1")
        nc.scalar.activation(sig1[:], shw_sb[:], FT.Sigmoid)
        rw_sb = pp.tile([P, ND, E], dt.float32, name="rw_sb")
        nc.scalar.dma_start(rw_sb[:], aps["rw"][:])
        rb_sb = pp.tile([1, E], dt.float32, name="rb_sb")
        nc.scalar.dma_start(rb_sb[:], aps["rb"][:, :])

        xtfs = []
        for i in range(3):
            xtf = pxT.tile([P, ND, P], dt.float32, tag="xtf", name=f"xtf{i}")
            nc.scalar.dma_start(xtf[:], aps["xtf"][:, i])
            xtfs.append(xtf)
        xtb = pxtb.tile([P, NT, ND, P], dt.bfloat16)
        nc.scalar.dma_start(xtb[:], aps["xtb"][:])
        sw1m_cur = {}
        sw1m = sw1p.tile([P, ND, 512], dt.bfloat16, tag="sw1m",
                         name="sw1m0")
        nc.scalar.dma_start(sw1m[:], aps["sw1"][0])
        sw1m_cur[0] = sw1m
        for i in range(3, NT):
            xtf = pxT.tile([P, ND, P], dt.float32, tag="xtf", name=f"xtf{i}")
            nc.scalar.dma_start(xtf[:], aps["xtf"][:, i])
            xtfs.append(xtf)
        for m5 in range(1, F2 // 512):
            sw1m = sw1p.tile([P, ND, 512], dt.bfloat16, tag="sw1m",
                             name=f"sw1m{m5}")
            nc.scalar.dma_start(sw1m[:], aps["sw1"][m5])
            sw1m_cur[m5] = sw1m

        c = _build_consts(nc, pp, pA, aps)
        c["rw_sb"] = rw_sb
        c["rb_sb"] = rb_sb
        c["sig1"] = sig1

        st = {"pos": [], "A": []}
        st["carry"] = pp.tile([E, 1], dt.float32, name="carry")
        nc.vector.memset(st["carry"][:], 0.0)
        st["rank_sb"] = pp.tile([E, T], dt.float32, name="rank_sb")
        st["d01"] = pp.tile([P, NT], dt.float32, name="d01")
        st["cw0"] = pp.tile([P, NT], dt.float32, name="cw0")
        st["cw1"] = pp.tile([P, NT], dt.float32, name="cw1")

        # ---- phase A: router + rank/compaction + shared-fc1 interleave ----
        for i in range(NT):
            _router_tile(nc, pA, pAps, pp, c, st, xtfs[i], i)
            _compaction_tile(nc, pA, pAps, pp, c, aps, st, i)
            _shared_fc1_group(nc, sw1p, ps3, c, aps, xtb, hsT, sw1m_cur, i)
        # batched top-2 softmax weights: 2 sigmoid ops, no table thrash
        nc.scalar.activation(st["cw0"][:], st["d01"][:], FT.Sigmoid)
        nc.scalar.activation(st["cw1"][:], st["d01"][:], FT.Sigmoid,
                             scale=-1.0)
        psig = pAps.tile([P, 8], dt.float32, tag="rtr", name="psig",
                         space="PSUM", bufs=2)
        nc.tensor.matmul(psig[:, 0:1], lhsT=c["onesf"][:, :],
                         rhs=c["sig1"][:, :], start=True, stop=True)
        nc.vector.tensor_copy(c["sig_bc"][:], psig[:, 0:1])

        # ---- dispatch: bucket-id readback + gathers (gpsimd ring) ----
        for g in range(8):
            nc.gpsimd.dma_start(
                idxw[g * 16:(g + 1) * 16, :],
                aps["bid"][:NIDXG, :].rearrange("(p c) one -> p (c one)",
                                                p=16))
        for e in range(5):
            nc.gpsimd.dma_gather(
                out_ap=gxe[e][:],
                in_ap=aps["xbf"][:, :],
                idxs_ap=idxw[:, e * NCH:e * NCH + GCAP // 16],
                num_idxs=GCAP, num_idxs_reg=GCAP, elem_size=D,
                transpose=True)

        # ---- remaining shared fc1 groups overlap the gathers ----
        for m in range(NT, 4 * (F2 // 512)):
            _shared_fc1_group(nc, sw1p, ps3, c, aps, xtb, hsT, sw1m_cur, m)

        ps3.release()
        pAps.release()
        pA.release()
        sw1p.release()
        pxtb.release()
        pxT.release()

        # ---- phase B: expert pairs ----
        hTp = tc.alloc_tile_pool(name="hTp", bufs=1)
        w1p = tc.alloc_tile_pool(name="w1p", bufs=3)
        w2p = tc.alloc_tile_pool(name="w2p", bufs=2)
        yevp = tc.alloc_tile_pool(name="yevp", bufs=4)
        pB = tc.alloc_tile_pool(name="pB", bufs=1)
        c["b2_sb"] = pB.tile([1, E, D], dt.bfloat16, name="b2_sb")
        nc.scalar.dma_start(c["b2_sb"][:],
                            aps["b2"].rearrange("e d2 -> e d2")[None, :, :])
        c["z512"] = pB.tile([1, 512], dt.bfloat16, name="z512")
        nc.vector.memset(c["z512"][:], 0.0)
        ps1 = tc.alloc_tile_pool(name="ps1", bufs=3, space="PSUM")
        ps2 = tc.alloc_tile_pool(name="ps2", bufs=1, space="PSUM")
        psL = tc.alloc_tile_pool(name="psL", bufs=1, space="PSUM")

        for pair in range(E // 2):
            eA, eB = 2 * pair, 2 * pair + 1
            hTA = hTp.tile([P, NF, CAP], dt.bfloat16, tag="hT0",
                           name=f"hT{eA}")
            hTB = hTp.tile([P, NF, CAP], dt.bfloat16, tag="hT1",
                           name=f"hT{eB}")
            def late_gather(done_e):
                e = done_e + 5
                if e < E:
                    gxe[e] = pgx.tile([P, ND, GCAP], dt.bfloat16,
                                      tag=f"gx{e - 5}", name=f"gx{e}")
                    nc.gpsimd.dma_gather(
                        out_ap=gxe[e][:],
                        in_ap=aps["xbf"][:, :],
                        idxs_ap=idxw[:, e * NCH:e * NCH + GCAP // 16],
                        num_idxs=GCAP, num_idxs_reg=GCAP, elem_size=D,
                        transpose=True)

            _fc1_expert(nc, w1p, ps1, c, aps, gxe, hTA, eA)
            late_gather(eA)
            _fc1_expert(nc, w1p, ps1, c, aps, gxe, hTB, eB)
            late_gather(eB)
            if eA == 0:
                nc.gpsimd.dma_start(sw2r[:], aps["sw2r"][:])
            _fc2_pair(nc, w2p, yevp, ps2, psL, c, aps, eA, eB, hTA, hTB)

        psL.release()
        ps2.release()
        ps1.release()
        pB.release()
        yevp.release()
        w2p.release()
        w1p.release()
        hTp.release()

        # ---- phase C: shared fc2 + combine + LayerNorm ----
        _shared_fc2_combine_ln(nc, tc, pp, c, aps, st, hsT, sw2r)

        pgx.release()
        sw2p.release()
        hsTp.release()
        pp.release()

    nc.compile()
    return nc


def _consts():
    iota8 = np.tile(np.arange(8, dtype=np.float32), (P, 1))
    iotat = np.arange(T, dtype=np.int16).reshape(NT, P).T.copy()
    tri = np.triu(np.ones((P, P), np.float32)).astype(ml_dtypes.bfloat16)
    ident = np.eye(P, dtype=np.float32)
    onesb = np.ones((1, P), dtype=ml_dtypes.bfloat16)
    onesf = np.ones((1, P), dtype=np.float32)
    tri8f = np.triu(np.ones((8, 8), np.float32))
    onescolb = np.ones((P, 1), dtype=ml_dtypes.bfloat16)
    return dict(iota8=iota8, iotat=iotat, tri=tri, ident=ident,
                onesb=onesb, onesf=onesf, tri8f=tri8f, onescolb=onescolb)


def _pack_w1(w1f):
    """[E, D, F] f32 -> [E, F//512, P, ND, 512] bf16 (fc1 SBUF tile layout)."""
    bf = ml_dtypes.bfloat16
    return np.ascontiguousarray(
        np.asarray(w1f, np.float32).astype(bf)
        .reshape(E, ND, P, F // 512, 512).transpose(0, 3, 2, 1, 4))


def _pack_w2(w2f):
    """[E, F, D] f32 -> [E, NF//4, P, 4, 2, 512] bf16 (joint d-halves)."""
    bf = ml_dtypes.bfloat16
    return np.ascontiguousarray(
        np.asarray(w2f, np.float32).astype(bf)
        .reshape(E, NF // 4, 4, P, 2, 512).transpose(0, 1, 3, 2, 4, 5))


def _pack_sw1(sw1f):
    """[D, F2] f32 -> [F2//512, P, ND, 512] bf16."""
    bf = ml_dtypes.bfloat16
    return np.ascontiguousarray(
        np.asarray(sw1f, np.float32).astype(bf)
        .reshape(ND, P, F2 // 512, 512).transpose(2, 1, 0, 3))


def _pack_sw2(sw2f):
    """[F2, D] f32 -> [P, NF2, D] bf16 (k-chunk-resident rhs layout)."""
    bf = ml_dtypes.bfloat16
    return np.ascontiguousarray(
        np.asarray(sw2f, np.float32).astype(bf)
        .reshape(NF2, P, D).transpose(1, 0, 2))


def _xt_layout(xc):
    """[T, D] -> [P, NT, ND, P]: out[p, i, k, q] = x[i*128+q, k*128+p]."""
    return np.ascontiguousarray(
        xc.reshape(NT, P, ND, P).transpose(3, 0, 2, 1))


def make_in_maps(inputs):
    """Build the 8 per-core input maps from the full problem inputs."""
    bf = ml_dtypes.bfloat16
    x = np.ascontiguousarray(
        np.asarray(inputs["hidden_states"], np.float32).reshape(-1, D))
    shared = dict(
        rw=np.ascontiguousarray(np.asarray(inputs["router_w"], np.float32)
                                .reshape(ND, P, E).transpose(1, 0, 2)),
        rb=np.asarray(inputs["router_b"], np.float32).reshape(1, E),
        w1=_pack_w1(inputs["w1"]),
        w2=_pack_w2(inputs["w2"]),
        b1=np.ascontiguousarray(np.asarray(inputs["b1"], np.float32)
                                .reshape(E, NF, P).transpose(2, 0, 1)),
        gate=np.ascontiguousarray(np.asarray(inputs["gate"], np.float32)
                                  .reshape(E, NF, P).transpose(2, 0, 1)),
        b2=np.asarray(inputs["b2"], np.float32).astype(bf),
        sw1=_pack_sw1(inputs["sw1"]),
        sb1=np.ascontiguousarray(np.asarray(inputs["sb1"], np.float32)
                                 .reshape(NF2, P).T),
        sgate=np.ascontiguousarray(np.asarray(inputs["sgate"], np.float32)
                                   .reshape(NF2, P).T),
        sw2r=_pack_sw2(inputs["sw2"]),
        sb2=np.asarray(inputs["sb2"], np.float32).astype(bf).reshape(1, D),
        shw=np.asarray(inputs["shared_weight"], np.float32).reshape(1, 1),
        lng=np.asarray(inputs["ln_g"], np.float32).reshape(1, D),
        lnb=np.asarray(inputs["ln_b"], np.float32).reshape(1, D),
        **_consts(),
    )
    maps = []
    for cix in range(NCORES):
        xc = np.ascontiguousarray(x[cix * T:(cix + 1) * T])
        xtf = _xt_layout(xc)
        maps.append({
            "xtf": xtf,
            "xtb": np.ascontiguousarray(xtf.astype(bf)),
            "xbf": np.ascontiguousarray(xc.astype(bf)),
            **shared,
        })
    return maps


def kernel(hidden_states, router_w, router_b, w1, b1, gate, w2, b2,
           sw1, sb1, sgate, sw2, sb2, shared_weight, ln_g, ln_b):
    global _PROGRAM
    if _PROGRAM is None:
        _PROGRAM = build_program()
    nc = _PROGRAM

    in_maps = make_in_maps(dict(
        hidden_states=hidden_states, router_w=router_w, router_b=router_b,
        w1=w1, b1=b1, gate=gate, w2=w2, b2=b2, sw1=sw1, sb1=sb1, sgate=sgate,
        sw2=sw2, sb2=sb2, shared_weight=shared_weight, ln_g=ln_g, ln_b=ln_b))
    res = run_bass_kernel_spmd(nc, in_maps, list(range(NCORES)))
    out = np.concatenate([res.results[c]["out"] for c in range(NCORES)], axis=0)
    return out.reshape(B, S, D).astype(np.float32)


if __name__ == "__main__":
    build_program()
    print("kernel program built OK")


# revision 28
# speedup vs baseline: 1.0362x; 1.0362x over previous
"""Trainium2 Bass kernel for nn_ExpertFFNEnsemble (MoE routing, 8 experts, top-2).

Strategy: data-parallel over tokens (8192 tokens -> 1024/core, 8 cores).
v4 restructure (v3 lessons: ACT table thrash, scalar-ring congestion,
PE-FIFO coupling of rank chain with shared-fc1):
  - phase A: all router tiles first (PE-light, DVE-driven), sigmoids
    BATCHED into 2 ACT ops (no sigmoid<->gelu table reloads), then
    rank/compaction+scatters, idxw readback + dispatch gathers on the
    gpsimd ring, THEN shared fc1 (58 us of PE) overlapping the gathers
  - expert phase: serial expert pairs; fc2 processes both d-halves per
    weight chunk so one hT LDWEIGHTS feeds two matmuls; w2 streams on
    the gpsimd ring (ACT only does fc1 gelu); 32-row fc2 leftovers pack
    2-way per j into one PSUM bank via tile_position col tiling
    (zero-matmul opens the bank, leftovers accumulate with start=False)
  - shared-expert fc2 LAST (sw2 SBUF-resident, j-outer) covering the
    per-tile combine + LayerNorm tail; LN uses DVE bn_stats/bn_aggr
    (mean+var in one pass, no ACT tables), ys scale on DVE
No cross-core communication; host shards tokens / packs weights and
concatenates per-core output slices.
"""

import sys

sys.path.insert(0, "/opt/trn_rl_repo")

import numpy as np
import ml_dtypes

import concourse.bass as bass
import concourse.mybir as mybir
import concourse.tile as tile
from concourse import bacc
from concourse.bass import IndirectOffsetOnAxis
from concourse.bass_utils import run_bass_kernel_spmd

P = 128
B, S, D, F = 4, 2048, 1024, 4096
F2 = F // 2
E = 8
NCORES = 8
T = (B * S) // NCORES           # 1024 tokens per core
NT = T // P                     # 8 token tiles
ND = D // P                     # 8 d-chunks
NF = F // P                     # 32 f-chunks
NF2 = F2 // P                   # 16 f2-chunks
CAP = 288                       # per-expert token capacity (2.25 x 128)
NIDX = E * CAP                  # 2304 bucket rows (%128 == 0)
NCH = CAP // 16                 # idx columns per expert (18)
TRASH = NIDX                    # overflow-redirect row
GCAP = 384                      # gather width per expert (3 x 128, padded)
NIDXG = (E - 1) * CAP + GCAP    # bid rows covered by padded gathers (2400)
NW = NIDXG // 16                # idxw row width (150)
BID_ROWS = ((NIDXG + P + 127) // P) * P   # bucket-id rows incl. trash, %128
YBK_ROWS = NIDX + P             # fc2 output rows incl. trash region
LN_EPS = 1e-5
FT = mybir.ActivationFunctionType
dt = mybir.dt
AX = mybir.AxisListType
OP = mybir.AluOpType

_PROGRAM = None


def _build_consts(nc, pp, pA, aps):
    c = {}
    c["iota8"] = pp.tile([P, 8], dt.float32, name="iota8")
    nc.scalar.dma_start(c["iota8"][:], aps["iota8"][:])
    c["tri"] = pp.tile([P, P], dt.bfloat16, name="tri")
    nc.scalar.dma_start(c["tri"][:], aps["tri"][:])
    c["ident"] = pp.tile([P, P], dt.float32, name="ident")
    nc.scalar.dma_start(c["ident"][:], aps["ident"][:])
    c["onesb"] = pp.tile([1, P], dt.bfloat16, name="onesb")
    nc.scalar.dma_start(c["onesb"][:], aps["onesb"][:])
    c["onesf"] = pp.tile([1, P], dt.float32, name="onesf")
    nc.scalar.dma_start(c["onesf"][:], aps["onesf"][:])
    c["onescolb"] = pp.tile([P, 1], dt.bfloat16, name="onescolb")
    nc.scalar.dma_start(c["onescolb"][:], aps["onescolb"][:])
    c["eps_t"] = pp.tile([P, 1], dt.float32, name="eps_t")
    nc.vector.memset(c["eps_t"][:], LN_EPS)

    c["gate_sb"] = pp.tile([P, E, NF], dt.float32, name="gate_sb")
    nc.scalar.dma_start(c["gate_sb"][:], aps["gate"][:])
    c["b1g_sb"] = pp.tile([P, E, NF], dt.float32, name="b1g_sb")
    b1_tmp = pA.tile([P, E, NF], dt.float32, tag="b1_tmp")
    nc.scalar.dma_start(b1_tmp[:], aps["b1"][:])
    nc.vector.tensor_mul(c["b1g_sb"][:], b1_tmp[:], c["gate_sb"][:])
    c["sg_sb"] = pp.tile([P, NF2], dt.float32, name="sg_sb")
    nc.scalar.dma_start(c["sg_sb"][:], aps["sgate"][:])
    c["sb1g_sb"] = pp.tile([P, NF2], dt.float32, name="sb1g_sb")
    sb1_tmp = pA.tile([P, NF2], dt.float32, tag="sb1_tmp")
    nc.scalar.dma_start(sb1_tmp[:], aps["sb1"][:])
    nc.vector.tensor_mul(c["sb1g_sb"][:], sb1_tmp[:], c["sg_sb"][:])

    # zero-init internal DRAM (bucket ids + ybk trash rows)
    zid = pA.tile([P, BID_ROWS // P], dt.int16, tag="zid")
    nc.vector.memset(zid[:], 0)
    nc.sync.dma_start(
        aps["bid"].rearrange("(p c) one -> p (c one)", p=P), zid[:])
    ztrash = pA.tile([P, D], dt.bfloat16, tag="ztrash")
    nc.vector.memset(ztrash[:], 0.0)
    nc.sync.dma_start(aps["ybk"][TRASH:TRASH + P, :], ztrash[:])

    c["sig_bc"] = pp.tile([P, 1], dt.float32, name="sig_bc")
    c["iotat_sb"] = pp.tile([P, NT], dt.int16, name="iotat_sb")
    nc.scalar.dma_start(c["iotat_sb"][:], aps["iotat"][:])
    return c


def _router_tile(nc, pA, pAps, pp, c, st, xtf, i):
    """Router logits + top-2 for token tile i (sigmoids deferred)."""
    pl = pAps.tile([P, 8], dt.float32, tag="rtr",
                   name=f"pl{i}", space="PSUM", bufs=2)
    for k in range(ND):
        nc.tensor.matmul(pl[:], lhsT=xtf[:, k, :],
                         rhs=c["rw_sb"][:, k, :],
                         start=(k == 0), stop=False)
    nc.tensor.matmul(pl[:], lhsT=c["onesf"][:, :], rhs=c["rb_sb"][:, :],
                     start=False, stop=True)
    vals = pA.tile([P, 8], dt.float32, tag="vals")
    idx = pA.tile([P, 8], dt.uint32, tag="idx")
    lt = pA.tile([P, 8], dt.float32, tag="lt")
    nc.vector.tensor_copy(lt[:], pl[:])
    nc.vector.max_with_indices(vals[:], idx[:], lt[:])

    nc.vector.tensor_sub(st["d01"][:, i:i + 1], vals[:, 0:1], vals[:, 1:2])

    ef = pA.tile([P, 2], dt.float32, tag=f"ef{i}")
    nc.vector.tensor_copy(ef[:], idx[:, 0:2])
    oh0 = pA.tile([P, 8], dt.float32, tag=f"oh0_{i}")
    oh1 = pA.tile([P, 8], dt.float32, tag=f"oh1_{i}")
    nc.vector.tensor_tensor(
        out=oh0[:], in0=ef[:, 0:1].to_broadcast([P, 8]),
        in1=c["iota8"][:], op=OP.is_equal)
    nc.vector.tensor_tensor(
        out=oh1[:], in0=ef[:, 1:2].to_broadcast([P, 8]),
        in1=c["iota8"][:], op=OP.is_equal)
    A = pA.tile([P, 8], dt.bfloat16, tag=f"A{i}")
    nc.vector.tensor_add(A[:], oh0[:], oh1[:])
    st["A"].append((A, ef, oh0, oh1))


def _compaction_tile(nc, pA, pAps, pp, c, aps, st, i):
    """Rank + bucket position + scatter for token tile i."""
    tsl = slice(i * P, (i + 1) * P)
    A, ef, oh0, oh1 = st["A"][i]

    # token-major inclusive rank: rk[t, e] = sum_{t'<=t} A[t', e], plus a
    # row-broadcast carry matmul -- no PE transposes in the chain
    rk = pAps.tile([P, E], dt.float32, tag="rtr2",
                   name=f"rk{i}", space="PSUM")
    nc.tensor.matmul(rk[:], lhsT=c["tri"][:], rhs=A[:],
                     start=True, stop=(i == 0))
    if i > 0:
        carr = pA.tile([1, E], dt.float32, tag="carr", name=f"carr{i}")
        nc.vector.tensor_copy(carr[:], st["carry_ps"][:])
        nc.tensor.matmul(rk[:], lhsT=c["onesf"][:, :], rhs=carr[:],
                         start=False, stop=True)
    # running per-expert totals (exclusive carry for the next tile)
    nc.tensor.matmul(st["carry_ps"][:], lhsT=c["onescolb"][:],
                     rhs=A[:], start=(i == 0), stop=(i == NT - 1))

    rank_t = pA.tile([P, E], dt.float32, tag="rank_t")
    nc.vector.tensor_copy(rank_t[:], rk[:])
    tmp = pA.tile([P, 8], dt.float32, tag="tmp")
    r0 = pA.tile([P, 1], dt.float32, tag="r0")
    r1 = pA.tile([P, 1], dt.float32, tag="r1")
    nc.vector.tensor_mul(tmp[:], oh0[:], rank_t[:])
    nc.vector.reduce_sum(r0[:], tmp[:], axis=AX.X)
    nc.vector.tensor_mul(tmp[:], oh1[:], rank_t[:])
    nc.vector.reduce_sum(r1[:], tmp[:], axis=AX.X)

    posf = pA.tile([P, 2], dt.float32, tag="posf")
    nc.vector.tensor_scalar(out=posf[:, 0:1], in0=ef[:, 0:1],
                            scalar1=float(CAP), scalar2=None, op0=OP.mult)
    nc.vector.tensor_scalar(out=posf[:, 1:2], in0=ef[:, 1:2],
                            scalar1=float(CAP), scalar2=None, op0=OP.mult)
    nc.vector.scalar_tensor_tensor(
        out=posf[:, 0:1], in0=r0[:], scalar=-1.0, in1=posf[:, 0:1],
        op0=OP.add, op1=OP.add)
    nc.vector.scalar_tensor_tensor(
        out=posf[:, 1:2], in0=r1[:], scalar=-1.0, in1=posf[:, 1:2],
        op0=OP.add, op1=OP.add)
    ovf = pA.tile([P, 2], dt.uint8, tag="ovf")
    nc.vector.tensor_scalar(out=ovf[:, 0:1], in0=r0[:], scalar1=float(CAP),
                            scalar2=None, op0=OP.is_gt)
    nc.vector.tensor_scalar(out=ovf[:, 1:2], in0=r1[:], scalar1=float(CAP),
                            scalar2=None, op0=OP.is_gt)
    trash = pA.tile([P, 2], dt.float32, tag="trash")
    nc.vector.memset(trash[:], float(TRASH))
    nc.vector.copy_predicated(posf[:], ovf[:], trash[:])
    pos_i = pp.tile([P, 2], dt.int32, tag=f"pos{i}")
    nc.vector.tensor_copy(pos_i[:], posf[:])
    st["pos"].append(pos_i)

    # transposed scatter offsets: sg = (pos % 16) * NW + (pos >> 4), so
    # the idxw readback is contiguous per partition (16-row wrap layout)
    div16 = pA.tile([P, 2], dt.int32, tag="div16")
    nc.vector.tensor_scalar(out=div16[:], in0=pos_i[:], scalar1=4,
                            scalar2=None, op0=OP.arith_shift_right)
    mod16 = pA.tile([P, 2], dt.int32, tag="mod16")
    nc.vector.tensor_scalar(out=mod16[:], in0=pos_i[:], scalar1=15,
                            scalar2=None, op0=OP.bitwise_and)
    sg = pA.tile([P, 2], dt.int32, tag="sg")
    nc.vector.tensor_scalar(out=sg[:], in0=mod16[:], scalar1=NW,
                            scalar2=None, op0=OP.mult)
    nc.vector.tensor_add(sg[:], sg[:], div16[:])
    for s in range(2):
        nc.gpsimd.indirect_dma_start(
            out=aps["bid"][:, :],
            out_offset=IndirectOffsetOnAxis(ap=sg[:, s:s + 1], axis=0),
            in_=c["iotat_sb"][:, i:i + 1], in_offset=None)


def _shared_fc1(nc, sw1p, ps3, c, aps, xtb, hsT, cur):
    """Shared expert fc1: per (m5, mm) one weight column feeds both
    512-token halves (shared LDWEIGHTS), 2 psum banks in flight."""
    for m5 in range(F2 // 512):
        sw1m = cur[m5]
        for mm in range(4):
            m = m5 * 4 + mm
            pms = [ps3.tile([P, 512], dt.float32, tag=f"pm3{n}",
                            name=f"pm3_{m}_{n}", space="PSUM")
                   for n in range(2)]
            for k in range(ND):
                for n in range(2):
                    nc.tensor.matmul(
                        pms[n][:], lhsT=sw1m[:, k, mm * P:(mm + 1) * P],
                        rhs=xtb[:, 4 * n:4 * n + 4, k, :],
                        start=(k == 0), stop=(k == ND - 1))
            for n in range(2):
                nc.scalar.activation(
                    hsT[:, m, n * 512:(n + 1) * 512], pms[n][:], FT.Gelu,
                    bias=c["sb1g_sb"][:, m:m + 1],
                    scale=c["sg_sb"][:, m:m + 1])


def _fc1_expert(nc, w1p, ps1, c, aps, gxe, hT, e):
    """Expert e fc1: 8 weight chunks x 4 f-tiles, gelu+gate into hT."""
    for m5 in range(F // 512):
        w1m = w1p.tile([P, ND, 512], dt.bfloat16, tag="w1m",
                       name=f"w1m_{e}_{m5}")
        nc.sync.dma_start(w1m[:], aps["w1"][e, m5])
        for mm in range(4):
            m = m5 * 4 + mm
            pm = ps1.tile([P, CAP], dt.float32, tag="pm",
                          name=f"pm_{e}_{m}", space="PSUM")
            for k in range(ND):
                nc.tensor.matmul(
                    pm[:], lhsT=w1m[:, k, mm * P:(mm + 1) * P],
                    rhs=gxe[e][:, k, 0:CAP],
                    start=(k == 0), stop=(k == ND - 1))
            nc.scalar.activation(
                hT[:, m, :], pm[:], FT.Gelu,
                bias=c["b1g_sb"][:, e, m:m + 1],
                scale=c["gate_sb"][:, e, m:m + 1])


def _fc2_pair(nc, w2p, yevp, ps2, psL, c, aps, eA, eB, hTA, hTB):
    """fc2 for an expert pair: both d-halves per weight chunk (one hT
    LDWEIGHTS feeds 2 matmuls); 4 shared full psum banks reused
    eA->eB; 32-row leftovers pack 2-way per j into one shared bank."""
    pysL = psL.tile([P, 512], dt.float32, tag="pysL",
                    name=f"pysL_{eA}", space="PSUM")
    nc.tensor.matmul(pysL[:, :], lhsT=c["z512"][:, 0:P],
                     rhs=c["z512"][:, :], start=True, stop=False)
    hTs = (hTA, hTB)
    for x, e in enumerate((eA, eB)):
        hT = hTs[x]
        pys = {}
        for t in range(2):
            for h in range(2):
                pys[(t, h)] = ps2.tile(
                    [P, 512], dt.float32, tag=f"py{t}{h}",
                    name=f"py_{e}_{t}{h}", space="PSUM")
        for kg in range(NF // 4):
            w2x = w2p.tile([P, 4, 2, 512], dt.bfloat16, tag="w2e",
                           name=f"w2e_{e}_{kg}")
            nc.scalar.dma_start(w2x[:], aps["w2"][e, kg])
            for j in range(4):
                k = kg * 4 + j
                stt = (k == 0)
                for t in range(2):
                    lhsT = hT[:, k, t * P:(t + 1) * P]
                    for h in range(2):
                        nc.tensor.matmul(
                            pys[(t, h)][:], lhsT=lhsT,
                            rhs=w2x[:, j, h, :], start=stt, stop=False)
                lhsTl = hT[:, k, 2 * P:2 * P + 32]
                for h in range(2):
                    lo = x * 64 + h * 32
                    nc.tensor.matmul(
                        pysL[lo:lo + 32, :], lhsT=lhsTl,
                        rhs=w2x[:, j, h, :], start=False, stop=False,
                        tile_position=(0, lo))
        # epilogue: bias matmul + evacuate full tiles of expert e
        for t in range(2):
            for h in range(2):
                hsl = slice(h * 512, (h + 1) * 512)
                nc.tensor.matmul(
                    pys[(t, h)][:], lhsT=c["onesb"][:, :],
                    rhs=c["b2_sb"][:, e, hsl], start=False, stop=True)
                yev = yevp.tile([P, 512], dt.bfloat16, tag="yev",
                                name=f"yev_{e}_{t}{h}")
                nc.vector.tensor_copy(yev[:], pys[(t, h)][:])
                nc.gpsimd.dma_start(
                    aps["ybk"][e * CAP + t * P:e * CAP + (t + 1) * P, hsl],
                    yev[:])
    # leftover bias + evacuation (all four regions live in pysL)
    for x, e in enumerate((eA, eB)):
        for h in range(2):
            hsl = slice(h * 512, (h + 1) * 512)
            lo = x * 64 + h * 32
            last = (x == 1 and h == 1)
            nc.tensor.matmul(
                pysL[lo:lo + 32, :], lhsT=c["onesb"][:, 0:32],
                rhs=c["b2_sb"][:, e, hsl], start=False, stop=last,
                tile_position=(0, lo))
    for x, e in enumerate((eA, eB)):
        yevL = yevp.tile([P, 512], dt.bfloat16, tag="yevL",
                         name=f"yevL_{e}")
        nc.vector.tensor_copy(yevL[:64, :], pysL[x * 64:x * 64 + 64, :])
        for h in range(2):
            hsl = slice(h * 512, (h + 1) * 512)
            nc.gpsimd.dma_start(
                aps["ybk"][e * CAP + 2 * P:e * CAP + 2 * P + 32, hsl],
                yevL[h * 32:h * 32 + 32, :])


def _shared_fc2_combine_ln(nc, tc, pp, c, aps, st, hsT, sw2r):
    """j-outer shared fc2 on PE; gathers + combine + LayerNorm pipelined
    per token tile underneath it. LN stats via DVE bn_stats/bn_aggr."""
    psC = tc.alloc_tile_pool(name="psC", bufs=2, space="PSUM")
    pCg = tc.alloc_tile_pool(name="pCg", bufs=6)
    pC = tc.alloc_tile_pool(name="pC", bufs=2)
    pCw = tc.alloc_tile_pool(name="pCw", bufs=1)

    lng_bc = pCw.tile([P, D], dt.float32, name="lng_bc")
    nc.scalar.dma_start(lng_bc[:], aps["lng"].to_broadcast([P, D]))
    lnb_bc = pCw.tile([P, D], dt.float32, name="lnb_bc")
    nc.scalar.dma_start(lnb_bc[:], aps["lnb"].to_broadcast([P, D]))
    sb2_sb = pCw.tile([1, D], dt.bfloat16, name="sb2_sb")
    nc.scalar.dma_start(sb2_sb[:], aps["sb2"][:, :])

    # issue all moe-output gathers up-front (gpsimd queue drains in order)
    g0s, g1s = [], []
    for i in range(NT):
        g0 = pCg.tile([P, D], dt.bfloat16, tag="g0", name=f"g0_{i}")
        g1 = pCg.tile([P, D], dt.bfloat16, tag="g1", name=f"g1_{i}")
        nc.gpsimd.indirect_dma_start(
            out=g0[:], out_offset=None, in_=aps["ybk"][:, :],
            in_offset=IndirectOffsetOnAxis(ap=st["pos"][i][:, 0:1], axis=0))
        nc.gpsimd.indirect_dma_start(
            out=g1[:], out_offset=None, in_=aps["ybk"][:, :],
            in_offset=IndirectOffsetOnAxis(ap=st["pos"][i][:, 1:2], axis=0))
        g0s.append(g0)
        g1s.append(g1)

    for i in range(NT):
        psY = [psC.tile([P, 512], dt.float32, tag=f"psY{n}",
                        name=f"psY_{i}_{n}", space="PSUM")
               for n in range(2)]
        for k in range(NF2):
            lhsT = hsT[:, k, i * P:(i + 1) * P]
            for n in range(2):
                nc.tensor.matmul(
                    psY[n][:], lhsT=lhsT,
                    rhs=sw2r[:, k, n * 512:(n + 1) * 512],
                    start=(k == 0), stop=False)
        ysi = pC.tile([P, D], dt.bfloat16, tag="ysi", name=f"ysi_{i}")
        for n in range(2):
            nsl = slice(n * 512, (n + 1) * 512)
            nc.tensor.matmul(
                psY[n][:], lhsT=c["onesb"][:, :],
                rhs=sb2_sb[:, nsl], start=False, stop=True)
            nc.vector.tensor_scalar(
                out=ysi[:, nsl], in0=psY[n][:],
                scalar1=c["sig_bc"][:, 0:1], scalar2=None, op0=OP.mult)

        # combine + LN for tile i (DVE-centric; ACT only does Sqrt)
        comb = pC.tile([P, D], dt.float32, tag="comb", name=f"comb_{i}")
        nc.vector.scalar_tensor_tensor(
            out=comb[:], in0=g0s[i][:], scalar=st["cw0"][:, i:i + 1],
            in1=ysi[:], op0=OP.mult, op1=OP.add)
        nc.vector.scalar_tensor_tensor(
            out=comb[:], in0=g1s[i][:], scalar=st["cw1"][:, i:i + 1],
            in1=comb[:], op0=OP.mult, op1=OP.add)
        stat6 = pC.tile([P, 2, 6], dt.float32, tag="stat6", name=f"st6_{i}")
        for n in range(2):
            nc.vector.bn_stats(stat6[:, n, :], comb[:, n * 512:(n + 1) * 512])
        mv = pC.tile([P, 2], dt.float32, tag="mv", name=f"mv_{i}")
        nc.vector.bn_aggr(mv[:], stat6[:])
        sd = pC.tile([P, 1], dt.float32, tag="sd", name=f"sd_{i}")
        nc.scalar.activation(sd[:], mv[:, 1:2], FT.Sqrt,
                             bias=c["eps_t"][:, 0:1])
        rinv = pC.tile([P, 1], dt.float32, tag="rinv", name=f"ri_{i}")
        nc.vector.reciprocal(rinv[:], sd[:])
        yc = pC.tile([P, D], dt.float32, tag="yc", name=f"yc_{i}")
        nc.vector.scalar_tensor_tensor(
            out=yc[:], in0=comb[:], scalar=mv[:, 0:1],
            in1=lng_bc[:], op0=OP.subtract, op1=OP.mult)
        o1 = pC.tile([P, D], dt.float32, tag="o1", name=f"o1_{i}")
        nc.vector.scalar_tensor_tensor(
            out=o1[:], in0=yc[:], scalar=rinv[:, 0:1],
            in1=lnb_bc[:], op0=OP.mult, op1=OP.add)
        nc.scalar.dma_start(aps["out"][i * P:(i + 1) * P, :], o1[:])

    pCw.release()
    pC.release()
    pCg.release()
    psC.release()


def build_program():
    nc = bacc.Bacc("TRN2", target_bir_lowering=False, debug=False,
                   num_devices=NCORES)

    def din(name, shape, dtype):
        return nc.dram_tensor(name, list(shape), dtype,
                              kind="ExternalInput").ap()

    aps = {
        "xtf": din("xtf", [P, NT, ND, P], dt.float32),
        "xtb": din("xtb", [P, NT, ND, P], dt.bfloat16),
        "xbf": din("xbf", [T, D], dt.bfloat16),
        "rw": din("rw", [P, ND, E], dt.float32),
        "rb": din("rb", [1, E], dt.float32),
        "w1": din("w1", [E, F // 512, P, ND, 512], dt.bfloat16),
        "w2": din("w2", [E, NF // 4, P, 4, 2, 512], dt.bfloat16),
        "b1": din("b1", [P, E, NF], dt.float32),
        "gate": din("gate", [P, E, NF], dt.float32),
        "b2": din("b2", [E, D], dt.bfloat16),
        "sw1": din("sw1", [F2 // 512, P, ND, 512], dt.bfloat16),
        "sb1": din("sb1", [P, NF2], dt.float32),
        "sgate": din("sgate", [P, NF2], dt.float32),
        "sw2r": din("sw2r", [P, NF2, D], dt.bfloat16),
        "sb2": din("sb2", [1, D], dt.bfloat16),
        "shw": din("shw", [1, 1], dt.float32),
        "lng": din("lng", [1, D], dt.float32),
        "lnb": din("lnb", [1, D], dt.float32),
        "iota8": din("iota8", [P, 8], dt.float32),
        "iotat": din("iotat", [P, NT], dt.int16),
        "tri": din("tri", [P, P], dt.bfloat16),
        "ident": din("ident", [P, P], dt.float32),
        "onesb": din("onesb", [1, P], dt.bfloat16),
        "onescolb": din("onescolb", [P, 1], dt.bfloat16),
        "onesf": din("onesf", [1, P], dt.float32),
    }
    aps["out"] = nc.dram_tensor("out", [T, D], dt.float32,
                                kind="ExternalOutput").ap()
    aps["bid"] = nc.dram_tensor("bid_i", [BID_ROWS, 1], dt.int16).ap()
    aps["ybk"] = nc.dram_tensor("ybk_i", [YBK_ROWS, D], dt.bfloat16).ap()

    with tile.TileContext(nc) as tc:
        # long-lived pools first (stack discipline)
        pp = tc.alloc_tile_pool(name="persist", bufs=1)
        hsTp = tc.alloc_tile_pool(name="hsTp", bufs=1)
        sw2p = tc.alloc_tile_pool(name="sw2p", bufs=1)
        pgx = tc.alloc_tile_pool(name="gx_pool", bufs=1)

        hsT = hsTp.tile([P, NF2, T], dt.bfloat16)
        sw2r = sw2p.tile([P, NF2, D], dt.bfloat16, name="sw2r")
        gxe = {}
        for e in range(5):
            gxe[e] = pgx.tile([P, ND, GCAP], dt.bfloat16, tag=f"gx{e}",
                              name=f"gx{e}")
        idxw = pgx.tile([P, NIDXG // 16], dt.int16)

        # ---- phase A pools
        pxT = tc.alloc_tile_pool(name="xtf_pool", bufs=4)
        pxtb = tc.alloc_tile_pool(name="xtb_pool", bufs=1)
        sw1p = tc.alloc_tile_pool(name="sw1p", bufs=3)
        pA = tc.alloc_tile_pool(name="pA", bufs=2)
        pAps = tc.alloc_tile_pool(name="pAps", bufs=1, space="PSUM")
        ps3 = tc.alloc_tile_pool(name="ps3", bufs=2, space="PSUM")

        # tiny critical DMAs first: shared-weight scalar + router weights
        shw_sb = pA.tile([1, 1], dt.float32, tag="shw", name="shw_sb")
        nc.scalar.dma_start(shw_sb[:], aps["shw"][:, :])
        sig1 = pA.tile([1, 1], dt.float32, tag="sig1", name="sig1")
        nc.scalar.activation(sig1[:], shw_sb[:], FT.Sigmoid)
        rw_sb = pp.tile([P, ND, E], dt.float32, name="rw_sb")
        nc.scalar.dma_start(rw_sb[:], aps["rw"][:])
        rb_sb = pp.tile([1, E], dt.float32, name="rb_sb")
        nc.scalar.dma_start(rb_sb[:], aps["rb"][:, :])

        xtfs = []
        for i in range(NT):
            xtf = pxT.tile([P, ND, P], dt.float32, tag="xtf", name=f"xtf{i}")
            nc.scalar.dma_start(xtf[:], aps["xtf"][:, i])
            xtfs.append(xtf)

        # bulk loads on the sync ring: xtb (shared fc1), sw1 stream
        xtb = pxtb.tile([P, NT, ND, P], dt.bfloat16)
        nc.sync.dma_start(xtb[:], aps["xtb"][:])
        sw1m_cur = {}
        for m5 in range(F2 // 512):
            sw1m = sw1p.tile([P, ND, 512], dt.bfloat16, tag="sw1m",
                             name=f"sw1m{m5}")
            nc.sync.dma_start(sw1m[:], aps["sw1"][m5])
            sw1m_cur[m5] = sw1m

        c = _build_consts(nc, pp, pA, aps)
        c["rw_sb"] = rw_sb
        c["rb_sb"] = rb_sb
        c["sig1"] = sig1

        st = {"pos": [], "A": []}
        st["carry_ps"] = pAps.tile([1, E], dt.float32, tag="carryps",
                                   name="carry_ps", space="PSUM")
        st["d01"] = pp.tile([P, NT], dt.float32, name="d01")
        st["cw0"] = pp.tile([P, NT], dt.float32, name="cw0")
        st["cw1"] = pp.tile([P, NT], dt.float32, name="cw1")

        # ---- phase A1: router for all tiles (DVE-driven, PE light) ----
        for i in range(NT):
            _router_tile(nc, pA, pAps, pp, c, st, xtfs[i], i)
        # batched top-2 softmax weights: 2 sigmoid ops, no table thrash
        nc.scalar.activation(st["cw0"][:], st["d01"][:], FT.Sigmoid)
        nc.scalar.activation(st["cw1"][:], st["d01"][:], FT.Sigmoid,
                             scale=-1.0)
        psig = pAps.tile([P, 8], dt.float32, tag="rtr", name="psig",
                         space="PSUM", bufs=2)
        nc.tensor.matmul(psig[:, 0:1], lhsT=c["onesf"][:, :],
                         rhs=c["sig1"][:, :], start=True, stop=True)
        nc.vector.tensor_copy(c["sig_bc"][:], psig[:, 0:1])

        # ---- phase A2: rank/compaction + scatters per tile ----
        for i in range(NT):
            _compaction_tile(nc, pA, pAps, pp, c, aps, st, i)

        # ---- dispatch: bucket-id readback + gathers (gpsimd ring) ----
        for g in range(8):
            nc.gpsimd.dma_start(
                idxw[g * 16:(g + 1) * 16, :],
                aps["bid"][:NIDXG, :].rearrange("(p c) one -> p (c one)",
                                                p=16))
        for e in range(5):
            nc.gpsimd.dma_gather(
                out_ap=gxe[e][:],
                in_ap=aps["xbf"][:, :],
                idxs_ap=idxw[:, e * NCH:e * NCH + GCAP // 16],
                num_idxs=GCAP, num_idxs_reg=GCAP, elem_size=D,
                transpose=True)

        # ---- shared fc1 (PE) overlapping the gathers ----
        _shared_fc1(nc, sw1p, ps3, c, aps, xtb, hsT, sw1m_cur)

        ps3.release()
        pAps.release()
        pA.release()
        sw1p.release()
        pxtb.release()
        pxT.release()

        # ---- phase B: expert pairs ----
        hTp = tc.alloc_tile_pool(name="hTp", bufs=1)
        w1p = tc.alloc_tile_pool(name="w1p", bufs=3)
        w2p = tc.alloc_tile_pool(name="w2p", bufs=2)
        yevp = tc.alloc_tile_pool(name="yevp", bufs=4)
        pB = tc.alloc_tile_pool(name="pB", bufs=1)
        c["b2_sb"] = pB.tile([1, E, D], dt.bfloat16, name="b2_sb")
        nc.scalar.dma_start(c["b2_sb"][:],
                            aps["b2"].rearrange("e d2 -> e d2")[None, :, :])
        c["z512"] = pB.tile([1, 512], dt.bfloat16, name="z512")
        nc.vector.memset(c["z512"][:], 0.0)
        ps1 = tc.alloc_tile_pool(name="ps1", bufs=3, space="PSUM")
        ps2 = tc.alloc_tile_pool(name="ps2", bufs=1, space="PSUM")
        psL = tc.alloc_tile_pool(name="psL", bufs=1, space="PSUM")

        for pair in range(E // 2):
            eA, eB = 2 * pair, 2 * pair + 1
            hTA = hTp.tile([P, NF, CAP], dt.bfloat16, tag="hT0",
                           name=f"hT{eA}")
            hTB = hTp.tile([P, NF, CAP], dt.bfloat16, tag="hT1",
                           name=f"hT{eB}")
            def late_gather(done_e):
                e = done_e + 5
                if e < E:
                    gxe[e] = pgx.tile([P, ND, GCAP], dt.bfloat16,
                                      tag=f"gx{e - 5}", name=f"gx{e}")
                    nc.gpsimd.dma_gather(
                        out_ap=gxe[e][:],
                        in_ap=aps["xbf"][:, :],
                        idxs_ap=idxw[:, e * NCH:e * NCH + GCAP // 16],
                        num_idxs=GCAP, num_idxs_reg=GCAP, elem_size=D,
                        transpose=True)

            _fc1_expert(nc, w1p, ps1, c, aps, gxe, hTA, eA)
            late_gather(eA)
            _fc1_expert(nc, w1p, ps1, c, aps, gxe, hTB, eB)
            late_gather(eB)
            if eA == 0:
                nc.sync.dma_start(sw2r[:], aps["sw2r"][:])
            _fc2_pair(nc, w2p, yevp, ps2, psL, c, aps, eA, eB, hTA, hTB)

        psL.release()
        ps2.release()
        ps1.release()
        pB.release()
        yevp.release()
        w2p.release()
        w1p.release()
        hTp.release()

        # ---- phase C: shared fc2 + combine + LayerNorm ----
        _shared_fc2_combine_ln(nc, tc, pp, c, aps, st, hsT, sw2r)

        pgx.release()
        sw2p.release()
        hsTp.release()
        pp.release()

    nc.compile()
    return nc


def _consts():
    iota8 = np.tile(np.arange(8, dtype=np.float32), (P, 1))
    iotat = np.arange(T, dtype=np.int16).reshape(NT, P).T.copy()
    tri = np.triu(np.ones((P, P), np.float32)).astype(ml_dtypes.bfloat16)
    ident = np.eye(P, dtype=np.float32)
    onesb = np.ones((1, P), dtype=ml_dtypes.bfloat16)
    onesf = np.ones((1, P), dtype=np.float32)
    onescolb = np.ones((P, 1), dtype=ml_dtypes.bfloat16)
    return dict(iota8=iota8, iotat=iotat, tri=tri, ident=ident,
                onesb=onesb, onesf=onesf, onescolb=onescolb)


def _pack_w1(w1f):
    """[E, D, F] f32 -> [E, F//512, P, ND, 512] bf16 (fc1 SBUF tile layout)."""
    bf = ml_dtypes.bfloat16
    return np.ascontiguousarray(
        np.asarray(w1f, np.float32).astype(bf)
        .reshape(E, ND, P, F // 512, 512).transpose(0, 3, 2, 1, 4))


def _pack_w2(w2f):
    """[E, F, D] f32 -> [E, NF//4, P, 4, 2, 512] bf16 (joint d-halves)."""
    bf = ml_dtypes.bfloat16
    return np.ascontiguousarray(
        np.asarray(w2f, np.float32).astype(bf)
        .reshape(E, NF // 4, 4, P, 2, 512).transpose(0, 1, 3, 2, 4, 5))


def _pack_sw1(sw1f):
    """[D, F2] f32 -> [F2//512, P, ND, 512] bf16."""
    bf = ml_dtypes.bfloat16
    return np.ascontiguousarray(
        np.asarray(sw1f, np.float32).astype(bf)
        .reshape(ND, P, F2 // 512, 512).transpose(2, 1, 0, 3))


def _pack_sw2(sw2f):
    """[F2, D] f32 -> [P, NF2, D] bf16 (k-chunk-resident rhs layout)."""
    bf = ml_dtypes.bfloat16
    return np.ascontiguousarray(
        np.asarray(sw2f, np.float32).astype(bf)
        .reshape(NF2, P, D).transpose(1, 0, 2))


def _xt_layout(xc):
    """[T, D] -> [P, NT, ND, P]: out[p, i, k, q] = x[i*128+q, k*128+p]."""
    return np.ascontiguousarray(
        xc.reshape(NT, P, ND, P).transpose(3, 0, 2, 1))


def make_in_maps(inputs):
    """Build the 8 per-core input maps from the full problem inputs."""
    bf = ml_dtypes.bfloat16
    x = np.ascontiguousarray(
        np.asarray(inputs["hidden_states"], np.float32).reshape(-1, D))
    shared = dict(
        rw=np.ascontiguousarray(np.asarray(inputs["router_w"], np.float32)
                                .reshape(ND, P, E).transpose(1, 0, 2)),
        rb=np.asarray(inputs["router_b"], np.float32).reshape(1, E),
        w1=_pack_w1(inputs["w1"]),
        w2=_pack_w2(inputs["w2"]),
        b1=np.ascontiguousarray(np.asarray(inputs["b1"], np.float32)
                                .reshape(E, NF, P).transpose(2, 0, 1)),
        gate=np.ascontiguousarray(np.asarray(inputs["gate"], np.float32)
                                  .reshape(E, NF, P).transpose(2, 0, 1)),
        b2=np.asarray(inputs["b2"], np.float32).astype(bf),
        sw1=_pack_sw1(inputs["sw1"]),
        sb1=np.ascontiguousarray(np.asarray(inputs["sb1"], np.float32)
                                 .reshape(NF2, P).T),
        sgate=np.ascontiguousarray(np.asarray(inputs["sgate"], np.float32)
                                   .reshape(NF2, P).T),
        sw2r=_pack_sw2(inputs["sw2"]),
        sb2=np.asarray(inputs["sb2"], np.float32).astype(bf).reshape(1, D),
        shw=np.asarray(inputs["shared_weight"], np.float32).reshape(1, 1),
        lng=np.asarray(inputs["ln_g"], np.float32).reshape(1, D),
        lnb=np.asarray(inputs["ln_b"], np.float32).reshape(1, D),
        **_consts(),
    )
    maps = []
    for cix in range(NCORES):
        xc = np.ascontiguousarray(x[cix * T:(cix + 1) * T])
        xtf = _xt_layout(xc)
        maps.append({
            "xtf": xtf,
            "xtb": np.ascontiguousarray(xtf.astype(bf)),
            "xbf": np.ascontiguousarray(xc.astype(bf)),
            **shared,
        })
    return maps


def kernel(hidden_states, router_w, router_b, w1, b1, gate, w2, b2,
           sw1, sb1, sgate, sw2, sb2, shared_weight, ln_g, ln_b):
    global _PROGRAM
    if _PROGRAM is None:
        _PROGRAM = build_program()
    nc = _PROGRAM

    in_maps = make_in_maps(dict(
        hidden_states=hidden_states, router_w=router_w, router_b=router_b,
        w1=w1, b1=b1, gate=gate, w2=w2, b2=b2, sw1=sw1, sb1=sb1, sgate=sgate,
        sw2=sw2, sb2=sb2, shared_weight=shared_weight, ln_g=ln_g, ln_b=ln_b))
    res = run_bass_kernel_spmd(nc, in_maps, list(range(NCORES)))
    out = np.concatenate([res.results[c]["out"] for c in range(NCORES)], axis=0)
    return out.reshape(B, S, D).astype(np.float32)


if __name__ == "__main__":
    build_program()
    print("kernel program built OK")
